# revision 1
# baseline (speedup 1.0000x reference)
"""DifferentialCausalAttention on 8 Trainium2 NeuronCores.

Sharding: 8 cores = 2 batches x 4 head-groups (tensor-parallel over heads).
Core c handles batch b = c // 4 and head-group g = c % 4:
  - query heads 8g..8g+7 (4 pairs), kv heads 4g..4g+3, lambda cols 4g..4g+3
  - W_O rows 512g..512g+511 -> partial output, host-summed over the 4 groups.

All matmuls run in float32r (full-rate fp32 mode on the PE).
Layouts on device: Q^T/K^T as [dh, L] (dh on partitions), V as [L, d],
attention computed transposed (S^T = [k, q]) so no P-transposes are needed.
"""
import os
from contextlib import ExitStack

import ml_dtypes
import numpy as np

import concourse.bass as bass
import concourse.mybir as mybir
import concourse.tile as tile
from concourse import bacc
from concourse.bass_utils import run_bass_kernel_spmd

F32 = mybir.dt.float32
F32R = mybir.dt.float32r
BF16 = mybir.dt.bfloat16

# dtype config: bf16 halves DMA and makes weight loads fast/overlappable,
# fp32r is ~20x more accurate. Toggles for experimentation.
ATT_BF16 = os.environ.get("ATT_BF16", "1") == "1"   # phase-2 S/ctx/rs operands
P1_BF16 = os.environ.get("P1_BF16", "1") == "1"     # phase-1 projection operands
P3_BF16 = os.environ.get("P3_BF16", "1") == "1"     # phase-3 Wo/diffT
DT_ATT = BF16 if ATT_BF16 else F32R
DT_P1 = BF16 if P1_BF16 else F32R
DT_P3 = BF16 if P3_BF16 else F32R
NP_P1 = ml_dtypes.bfloat16 if P1_BF16 else np.float32
NP_ATT = ml_dtypes.bfloat16 if ATT_BF16 else np.float32
NP_P3 = ml_dtypes.bfloat16 if P3_BF16 else np.float32

B, L, D, NH = 2, 2048, 2048, 16
DH = D // NH            # 128
G = 4                   # head groups (cores per batch)
NKV = NH // G           # kv heads per core = 4
NQ = 2 * NKV            # query heads per core = 8
CQK = NQ * DH + NKV * DH  # 1536 projection cols (Q then K)
CT = CQK // 128         # 12 column tiles (0-7 Q heads, 8-11 K heads)
DC = D // 128           # 16 contraction chunks
LCH = L // 512          # 4 L-chunks
LT = L // 128           # 16 L-tiles / q-tiles
SCALE = 1.0 / float(np.sqrt(DH))
ROPE_BASE = 10000.0


def build_kernel() -> bacc.Bacc:
    nc = bacc.Bacc("TRN2", target_bir_lowering=False, debug=False)

    xT = nc.dram_tensor("xT", [D, L], DT_P1, kind="ExternalInput")
    Wqk = nc.dram_tensor("Wqk", [D, CQK], DT_P1, kind="ExternalInput")
    Wv = nc.dram_tensor("Wv", [D, NKV * DH], DT_P1, kind="ExternalInput")
    Wl = nc.dram_tensor("Wl", [D, 128], DT_P1, kind="ExternalInput")
    blv = nc.dram_tensor("blv", [128, 1], F32, kind="ExternalInput")
    Wo = nc.dram_tensor("Wo", [NKV * DH, D], DT_P3, kind="ExternalInput")
    cosT = nc.dram_tensor("cosT", [DH, L], F32, kind="ExternalInput")
    sinTs = nc.dram_tensor("sinTs", [DH, L], F32, kind="ExternalInput")
    maskT = nc.dram_tensor("maskT", [128, 256], DT_ATT, kind="ExternalInput")
    onesin = nc.dram_tensor("onesin", [128, 128], F32R, kind="ExternalInput")
    outT = nc.dram_tensor("outT", [D, L], F32, kind="ExternalOutput")
    dbg = None
    if os.environ.get("KDBG") == "1":
        dbg = nc.dram_tensor("dbg", [20, L], F32, kind="ExternalOutput")

    with ExitStack() as ctx:
        tc = ctx.enter_context(tile.TileContext(nc))

        persist = ctx.enter_context(tc.tile_pool(name="persist", bufs=1))
        dram = ctx.enter_context(tc.tile_pool(name="dram", bufs=1, space="DRAM"))

        # ---- persistent tiles (loads deferred so phase-1 tiles go first) ----
        mask_sb = persist.tile([128, 256], DT_ATT)
        ones_sb = persist.tile([128, 128], F32R)
        ones_att = persist.tile([128, 1], DT_ATT)
        bl_sb = persist.tile([128, 1], F32)
        wo_sb = persist.tile([128, NKV, D], DT_P3)
        lam_sb = persist.tile([NKV, L], F32)          # sigmoid(x@Wl+bl), row per kv head
        diffT = persist.tile([128, NKV, L], DT_P3)     # (ctx0 - lam*ctx1)^T per head

        # DRAM scratch between phases
        qkT_d = dram.tile([CT, 128, L], DT_ATT)         # Q^T/K^T after RoPE
        v_d = dram.tile([L, NKV * DH], DT_ATT)          # V in [L, d] layout

        # ================= Phase 1: projections + RoPE =================
        with tc.tile_pool(name="ph1", bufs=1) as ph1, \
                tc.tile_pool(name="ps1", bufs=1, space="PSUM") as ps1:
            xTr = xT.rearrange("(dc p) l -> p dc l", p=128)
            wqkr = Wqk.rearrange("(dc p) c -> p dc c", p=128)
            wv_sb = ph1.tile([128, DC, NKV * DH], DT_P1)
            wl_sb = ph1.tile([128, DC, 128], DT_P1)

            for lch in range(LCH):
                ls = slice(lch * 512, (lch + 1) * 512)
                xs = ph1.tile([128, DC, 512], DT_P1, name="xs", tag="xs", bufs=2)
                for dc in range(DC):
                    nc.sync.dma_start(xs[:, dc, :], xTr[:, dc, ls])
                cos_sb = ph1.tile([128, 512], F32, name="cos_sb", tag="cos", bufs=2)
                sin_sb = ph1.tile([128, 512], F32, name="sin_sb", tag="sin", bufs=2)
                if os.environ.get("NO_TAB") != "1":
                    nc.sync.dma_start(cos_sb[:], cosT[:, ls])
                    nc.sync.dma_start(sin_sb[:], sinTs[:, ls])

                # --- Q^T / K^T column tiles + RoPE ---
                for ct in range(CT):
                    wt = ph1.tile([128, DC, 128], DT_P1, name="wt", tag="wt", bufs=3)
                    nc.sync.dma_start(wt[:], wqkr[:, :, ct * 128:(ct + 1) * 128])
                    qk_ps = ps1.tile([128, 512], F32, name="qk_ps", tag="mm512", bufs=4)
                    for dc in range(DC):
                        nc.tensor.matmul(
                            qk_ps[:], wt[:, dc, :], xs[:, dc, :],
                            start=(dc == 0), stop=(dc == DC - 1),
                        )
                    # RoPE: qr = qk*cos + rot(qk)*sin_signed
                    qkr_sb = ph1.tile([128, 512], DT_ATT, name="qkr_sb", tag="qkr", bufs=3)
                    if os.environ.get("NO_ROPE") == "1":
                        nc.scalar.copy(qkr_sb[:], qk_ps[:])
                    else:
                        rot = ph1.tile([128, 512], F32, name="rot", tag="rot", bufs=2)
                        nc.scalar.copy(rot[0:64, :], qk_ps[64:128, :])
                        nc.scalar.copy(rot[64:128, :], qk_ps[0:64, :])
                        t1 = ph1.tile([128, 512], F32, name="t1", tag="t1", bufs=2)
                        nc.vector.tensor_mul(t1[:], qk_ps[:], cos_sb[:])
                        t2 = ph1.tile([128, 512], F32, name="t2", tag="t2", bufs=2)
                        nc.vector.tensor_mul(t2[:], rot[:], sin_sb[:])
                        nc.vector.tensor_add(qkr_sb[:], t1[:], t2[:])
                    nc.sync.dma_start(qkT_d[ct, :, ls], qkr_sb[:])
                    if lch == 0 and ct == 0:
                        # big/aux loads ride behind the first column tile
                        nc.sync.dma_start(
                            wv_sb[:], Wv.rearrange("(dc p) c -> p dc c", p=128)
                        )
                        nc.sync.dma_start(
                            wl_sb[:], Wl.rearrange("(dc p) c -> p dc c", p=128)
                        )
                        nc.sync.dma_start(bl_sb[:], blv[:, :])
                    if dbg is not None and ct == 0 and os.environ.get("NO_ROPE") != "1":
                        for di, srcap in enumerate(
                            (qk_ps[0:1, :], cos_sb[0:1, :], sin_sb[0:1, :], rot[0:1, :])
                        ):
                            db = ph1.tile([1, 512], F32, name="db", tag=f"db{di}", bufs=2)
                            nc.vector.tensor_copy(db[:], srcap)
                            nc.sync.dma_start(dbg[NKV + 2 + di:NKV + 3 + di, ls], db[:])

                # --- V tiles ---
                for lt in ([] if os.environ.get("NO_V") == "1" else range(4)):
                    v_ps = ps1.tile([128, 512], F32, name="v_ps", tag="mm512", bufs=4)
                    for dc in range(DC):
                        nc.tensor.matmul(
                            v_ps[:], xs[:, dc, lt * 128:(lt + 1) * 128], wv_sb[:, dc, :],
                            start=(dc == 0), stop=(dc == DC - 1),
                        )
                    v_sb = ph1.tile([128, 512], DT_ATT, name="v_sb", tag="v_sb", bufs=2)
                    nc.scalar.copy(v_sb[:], v_ps[:])
                    nc.sync.dma_start(
                        v_d[lch * 512 + lt * 128: lch * 512 + (lt + 1) * 128, :], v_sb[:]
                    )

                # --- lambda ---
                if os.environ.get("NO_LAM") == "1":
                    continue
                lam_ps = ps1.tile([128, 512], F32, name="lam_ps", tag="mm512", bufs=4)
                for dc in range(DC):
                    nc.tensor.matmul(
                        lam_ps[:], wl_sb[:, dc, :], xs[:, dc, :],
                        start=(dc == 0), stop=(dc == DC - 1),
                    )
                nc.scalar.activation(
                    lam_sb[:, ls], lam_ps[0:NKV, :],
                    mybir.ActivationFunctionType.Sigmoid, bias=bl_sb[0:NKV, 0:1],
                )

        if dbg is not None:
            if os.environ.get("NO_LAM") != "1":
                nc.sync.dma_start(dbg[0:NKV, :], lam_sb[:, :])
            for cti in range(CT):
                nc.sync.dma_start(dbg[NKV + cti:NKV + cti + 1, :], qkT_d[cti, 0:1, :].bitcast(F32))
        trunc = os.environ.get("KTRUNC") == "1"
        # ================= Phase 2: causal attention per head pair =================
        # Two q-tiles (a "superblock": A=2sb, B=2sb+1) are processed at once so
        # every moving operand is 512 wide: columns ordered (qtile, head, l) =
        # [A.h0 | A.h1 | B.h0 | B.h1]. Per k-chunk: one S matmul [128,512], one
        # exp, one ctx matmul, one rowsum matmul. Chunk kc==A is full width but
        # masks its [0:256] half; chunk kc==B covers only [256:512]. ctx/rs for
        # chunk kc are emitted after the S matmul of chunk kc+2 (PE never waits
        # on ACT); the normalization tail is deferred into the next superblock.
        with tc.tile_pool(name="ph2", bufs=1) as ph2, \
                tc.tile_pool(name="ps2", bufs=1, space="PSUM") as ps2:
            v_r = v_d.rearrange("(kc pp) d -> pp kc d", pp=128)
            pend_norm = []

            def emit_block(st):
                ctx_ps, rs_ps, e_sb, j, kc, qtB, off, wid = st
                nc.tensor.matmul(
                    ctx_ps[:, off:off + wid], vp_sb[:, kc, :], e_sb[:, j, off:off + wid],
                    start=(kc == 0), stop=(kc == qtB), skip_group_check=True,
                )
                nc.tensor.matmul(
                    rs_ps[0:1, off:off + wid], ones_att[:, 0:1], e_sb[:, j, off:off + wid],
                    start=(kc == 0), stop=(kc == qtB), skip_group_check=True,
                )

            def emit_norm(st):
                ctx_ps, rs_sb, qtA, p_, lam0_ = st
                recip = ph2.tile([1, 2, 256], F32, name="recip", tag="recip", bufs=2)
                nc.vector.reciprocal_approx_fast(
                    recip.rearrange("p t l -> p (t l)"), rs_sb[:]
                )
                cs = ph2.tile([1, 2, 256], F32R, name="cs", tag="cs", bufs=2)
                nc.vector.tensor_copy(cs[:, :, 0:128], recip[:, :, 0:128])
                nc.vector.tensor_mul(
                    cs[:, :, 128:256], recip[:, :, 128:256],
                    lam0_[:, qtA * 128:(qtA + 2) * 128].rearrange(
                        "p (t l) -> p t l", t=2
                    ),
                )
                b_ps = ps2.tile([128, 512], F32, name="b_ps", tag="bps", bufs=1)
                nc.tensor.matmul(
                    b_ps[:], ones_sb[0:1, :], cs.rearrange("p t l -> p (t l)"),
                    start=True, stop=True,
                )
                b_sb = ph2.tile([128, 2, 256], F32, name="b_sb", tag="bsb", bufs=2)
                nc.vector.tensor_copy(b_sb.rearrange("p t l -> p (t l)"), b_ps[:])
                ctx3 = ctx_ps.rearrange("p (t l) -> p t l", t=2)
                t0 = ph2.tile([128, 2, 128], F32, name="t0", tag="t0", bufs=2)
                nc.vector.tensor_mul(t0[:], ctx3[:, :, 0:128], b_sb[:, :, 0:128])
                t1b = ph2.tile([128, 2, 128], F32, name="t1b", tag="t1b", bufs=2)
                nc.vector.tensor_mul(t1b[:], ctx3[:, :, 128:256], b_sb[:, :, 128:256])
                nc.vector.tensor_sub(
                    diffT[:, p_, qtA * 128:(qtA + 2) * 128],
                    t0.rearrange("p t l -> p (t l)"),
                    t1b.rearrange("p t l -> p (t l)"),
                )

            if not trunc:
                nc.sync.dma_start(mask_sb[:], maskT[:, :])
                nc.sync.dma_start(ones_sb[:], onesin[:, :])
                nc.vector.tensor_copy(ones_att[:], ones_sb[:, 0:1])
                nc.sync.dma_start(wo_sb[:], Wo.rearrange("(p d) o -> d p o", d=128))
            for p in ([] if trunc else range(NKV)):
                # [128, 16(t), 2(h), 128(l)] so superblock slices are contiguous
                qt_sb = ph2.tile([128, LT, 2, 128], DT_ATT, name="qt_sb", tag="qt", bufs=2)
                nc.sync.dma_start(
                    qt_sb[:],
                    qkT_d[2 * p:2 * p + 2, :, :].rearrange(
                        "h p (t l) -> p t h l", t=LT
                    ),
                )
                lam0 = ph2.tile([1, L], F32, name="lam0", tag="lam0", bufs=2)
                nc.gpsimd.dma_start(lam0[:], lam_sb[p:p + 1, :])
                kt_sb = ph2.tile([128, L], DT_ATT, name="kt_sb", tag="kt", bufs=2)
                nc.sync.dma_start(kt_sb[:], qkT_d[NQ + p, :, :])
                vp_sb = ph2.tile([128, LT, 128], DT_ATT, name="vp_sb", tag="vp", bufs=2)
                nc.sync.dma_start(vp_sb[:], v_r[:, :, p * 128:(p + 1) * 128])

                for sb in range(LT // 2):
                    qtA, qtB = 2 * sb, 2 * sb + 1
                    ctx_ps = ps2.tile([128, 512], F32, name="ctx_ps", tag="ctx", bufs=2)
                    rs_ps = ps2.tile([1, 512], F32, name="rs_ps", tag="small", bufs=1)
                    pend = []
                    # chunk groups of 2 sharing one 2-bank S tile; exp per group
                    groups = []
                    kcs = list(range(qtB + 1))
                    for gi in range(0, len(kcs), 2):
                        groups.append(kcs[gi:gi + 2])
                    for gk, grp in enumerate(groups):
                        s_ps = ps2.tile([128, 2, 512], F32, name="s_ps", tag="s2", bufs=2)
                        segs = []
                        for j, kc in enumerate(grp):
                            off, wid = (256, 256) if kc == qtB else (0, 512)
                            rhs = (qt_sb[:, qtA:qtA + 2, :, :] if wid == 512
                                   else qt_sb[:, qtB, :, :])
                            nc.tensor.matmul(
                                s_ps[:, j, off:off + wid],
                                kt_sb[:, kc * 128:(kc + 1) * 128],
                                rhs,
                                start=True, stop=True, skip_group_check=True,
                            )
                            segs.append((j, kc, off, wid))
                        if gk == 0 and len(pend_norm) >= 1:
                            emit_norm(pend_norm.pop(0))  # deferred tail
                        while len(pend) >= 2:
                            emit_block(pend.pop(0))
                        e_sb = ph2.tile([128, 2, 512], DT_ATT, name="e_sb", tag="e", bufs=3)
                        lo = segs[0][2]
                        hi = segs[-1][2] + segs[-1][3]
                        if len(segs) == 2 and segs[0][3] == 512 and segs[1][3] == 512:
                            nc.scalar.activation(
                                e_sb.rearrange("p a b -> p (a b)"),
                                s_ps.rearrange("p a b -> p (a b)"),
                                mybir.ActivationFunctionType.Exp, scale=SCALE,
                            )
                        else:
                            for j, kc, off, wid in segs:
                                nc.scalar.activation(
                                    e_sb[:, j, off:off + wid], s_ps[:, j, off:off + wid],
                                    mybir.ActivationFunctionType.Exp, scale=SCALE,
                                )
                        for j, kc, off, wid in segs:
                            if kc == qtA:
                                nc.vector.tensor_mul(
                                    e_sb[:, j, 0:256], e_sb[:, j, 0:256], mask_sb[:]
                                )
                            elif kc == qtB:
                                nc.vector.tensor_mul(
                                    e_sb[:, j, 256:512], e_sb[:, j, 256:512], mask_sb[:]
                                )
                            pend.append((ctx_ps, rs_ps, e_sb, j, kc, qtB, off, wid))
                    for st in pend:
                        emit_block(st)
                    # eager rowsum copy frees the PSUM bank promptly (DVE)
                    rs_sb = ph2.tile([1, 512], F32, name="rs_sb", tag="rs_sb", bufs=3)
                    nc.vector.tensor_copy(rs_sb[:], rs_ps[:])
                    pend_norm.append((ctx_ps, rs_sb, qtA, p, lam0))
            for st in pend_norm:
                emit_norm(st)

        # ================= Phase 3: output projection =================
        with tc.tile_pool(name="ph3", bufs=1) as ph3, \
                tc.tile_pool(name="ps3", bufs=1, space="PSUM") as ps3:
            for ot in ([] if trunc else range(LT)):
                for qch in range(LCH):
                    o_ps = ps3.tile([128, 512], F32, name="o_ps", tag="mm512", bufs=4)
                    for p in range(NKV):
                        nc.tensor.matmul(
                            o_ps[:],
                            wo_sb[:, p, ot * 128:(ot + 1) * 128],
                            diffT[:, p, qch * 512:(qch + 1) * 512],
                            start=(p == 0), stop=(p == NKV - 1),
                        )
                    o_sb = ph3.tile([128, 512], F32, name="o_sb", tag="osb", bufs=4)
                    nc.scalar.copy(o_sb[:], o_ps[:])
                    nc.sync.dma_start(
                        outT[ot * 128:(ot + 1) * 128, qch * 512:(qch + 1) * 512], o_sb[:]
                    )

    nc.finalize()
    return nc


def _host_tables():
    half = DH // 2
    inv_freq = 1.0 / (ROPE_BASE ** (np.arange(0, half, dtype=np.float64) * 2.0 / DH))
    freqs = np.arange(L, dtype=np.float64)[:, None] * inv_freq[None, :]  # [L, half]
    emb = np.concatenate([freqs, freqs], axis=-1)  # [L, DH]
    cosT = np.ascontiguousarray(np.cos(emb).T.astype(np.float32))  # [DH, L]
    sinT = np.sin(emb).T.astype(np.float32)
    sinTs = np.concatenate([-sinT[:half], sinT[half:]], axis=0)
    sinTs = np.ascontiguousarray(sinTs.astype(np.float32))
    tri = np.triu(np.ones((128, 128), dtype=np.float32))  # keep k' <= q'
    maskT = np.ascontiguousarray(np.concatenate([tri, tri], axis=1))
    ones = np.ones((128, 128), dtype=np.float32)
    return cosT, sinTs, maskT, ones


_NC_CACHE = []


def kernel(x, Wq, Wk, Wv, Wl, bl, Wo):
    x = np.asarray(x, dtype=np.float32)
    Wq = np.asarray(Wq, dtype=np.float32)
    Wk = np.asarray(Wk, dtype=np.float32)
    Wv = np.asarray(Wv, dtype=np.float32)
    Wl = np.asarray(Wl, dtype=np.float32)
    bl = np.asarray(bl, dtype=np.float32)
    Wo = np.asarray(Wo, dtype=np.float32)

    cosT, sinTs, maskT, ones = _host_tables()
    Wq3 = Wq.reshape(D, 2 * NH, DH)
    Wk3 = Wk.reshape(D, NH, DH)

    in_maps = []
    for c in range(8):
        b, g = divmod(c, G)
        wq_s = Wq3[:, 8 * g:8 * g + NQ, :].reshape(D, NQ * DH)
        wk_s = Wk3[:, G * g:G * g + NKV, :].reshape(D, NKV * DH)
        in_maps.append({
            "xT": np.ascontiguousarray(x[b].T).astype(NP_P1),
            "Wqk": np.ascontiguousarray(np.concatenate([wq_s, wk_s], axis=1)).astype(NP_P1),
            "Wv": np.ascontiguousarray(Wv[:, DH * G * g:DH * G * g + NKV * DH]).astype(NP_P1),
            "Wl": np.ascontiguousarray(np.pad(Wl[:, G * g:G * g + NKV], ((0, 0), (0, 128 - NKV)))).astype(NP_P1),
            "blv": np.ascontiguousarray(np.pad(bl[G * g:G * g + NKV], (0, 128 - NKV)).reshape(128, 1)),
            "Wo": np.ascontiguousarray(Wo[512 * g:512 * (g + 1), :]).astype(NP_P3),
            "cosT": cosT,
            "sinTs": sinTs,
            "maskT": maskT.astype(NP_ATT),
            "onesin": ones,
        })

    if not _NC_CACHE:
        _NC_CACHE.append(build_kernel())
    nc = _NC_CACHE[0]
    res = run_bass_kernel_spmd(nc, in_maps, core_ids=list(range(8)))

    out = np.empty((B, L, D), dtype=np.float32)
    for b in range(B):
        acc = res.results[4 * b]["outT"].copy()
        for g in range(1, G):
            acc += res.results[4 * b + g]["outT"]
        out[b] = acc.T
    return out



# revision 4
# speedup vs baseline: 1.1945x; 1.1945x over previous
"""DifferentialCausalAttention on 8 Trainium2 NeuronCores.

Sharding: 8 cores = 2 batches x 4 head-groups (tensor-parallel over heads).
Core c handles batch b = c // 4 and head-group g = c % 4:
  - query heads 8g..8g+7 (4 pairs), kv heads 4g..4g+3, lambda cols 4g..4g+3
  - W_O rows 512g..512g+511 -> partial output, host-summed over the 4 groups.

v2 design (vs baseline):
  - Q^T/K^T/V/diffT stay SBUF-resident between phases (no DRAM round-trip).
  - RoPE: one ACT copy PSUM->SBUF, partition-rotation via SBUF-SBUF DMA on the
    gpsimd queue, then bf16 DVE mul/mul/add (2x mode).
  - Attention rowsum moved off the PE: DVE accumulates exp tiles into rs_acc
    (bf16), one ones-matmul per (head, superblock) reduces partitions.
  - Output projection (Wo) matmuls are interleaved into the attention loop as
    filler work per 512-wide L chunk, so PE bubbles from the S->exp->ctx
    dependency chain are filled and there is no separate phase-3 window.
  - Startup DMA ordering: first weight tile + first x chunk land before the
    bulk loads, so the first matmul issues within a few us.
"""
import os
from collections import deque
from contextlib import ExitStack

import ml_dtypes
import numpy as np

import concourse.bass as bass
import concourse.mybir as mybir
import concourse.tile as tile
from concourse import bacc
from concourse.bass_utils import run_bass_kernel_spmd

F32 = mybir.dt.float32
F32R = mybir.dt.float32r
BF16 = mybir.dt.bfloat16

B, L, D, NH = 2, 2048, 2048, 16
DH = D // NH            # 128
G = 4                   # head groups (cores per batch)
NKV = NH // G           # kv heads per core = 4
NQ = 2 * NKV            # query heads per core = 8
CQK = NQ * DH + NKV * DH  # 1536 projection cols (Q then K)
CT = CQK // 128         # 12 column tiles (0-7 Q heads, 8-11 K heads)
DC = D // 128           # 16 contraction chunks
LCH = L // 512          # 4 L-chunks
LT = L // 128           # 16 L-tiles / q-tiles
SB = LT // 2            # 8 superblocks
SCALE = 1.0 / float(np.sqrt(DH))
ROPE_BASE = 10000.0


def build_kernel() -> bacc.Bacc:
    nc = bacc.Bacc("TRN2", target_bir_lowering=False, debug=False)

    xT = nc.dram_tensor("xT", [D, L], BF16, kind="ExternalInput")
    Wqk = nc.dram_tensor("Wqk", [D, CQK], BF16, kind="ExternalInput")
    Wv = nc.dram_tensor("Wv", [D, NKV * DH], BF16, kind="ExternalInput")
    Wl = nc.dram_tensor("Wl", [D, 128], BF16, kind="ExternalInput")
    blv = nc.dram_tensor("blv", [128, 1], F32, kind="ExternalInput")
    Wo = nc.dram_tensor("Wo", [NKV * DH, D], BF16, kind="ExternalInput")
    cosT = nc.dram_tensor("cosT", [DH, L], BF16, kind="ExternalInput")
    sinTs = nc.dram_tensor("sinTs", [DH, L], BF16, kind="ExternalInput")
    maskT = nc.dram_tensor("maskT", [128, 256], BF16, kind="ExternalInput")
    onesin = nc.dram_tensor("onesin", [128, 128], F32R, kind="ExternalInput")
    onesb = nc.dram_tensor("onesb", [128, 1], BF16, kind="ExternalInput")
    outT = nc.dram_tensor("outT", [D, L], F32, kind="ExternalOutput")

    with ExitStack() as ctx:
        tc = ctx.enter_context(tile.TileContext(nc))

        persist = ctx.enter_context(tc.tile_pool(name="persist", bufs=1))

        # ---- persistent SBUF tensors ----
        qres = persist.tile([128, LT, NQ, 128], BF16)   # Q^T roped, (t, h, l)
        kres = persist.tile([128, NKV, L], BF16)        # K^T roped
        vres = persist.tile([128, LT, NKV, 128], BF16)  # V, l on partitions
        diffT = persist.tile([128, NKV, L], BF16)       # (ctx0-lam*ctx1)/rs ^T
        wo_sb = persist.tile([128, NKV, D], BF16)
        wv_sb = persist.tile([128, DC, NKV * DH], BF16)
        wl_sb = persist.tile([128, DC, 128], BF16)
        cos_sb = persist.tile([128, L], BF16)
        sin_sb = persist.tile([128, L], BF16)
        lamT = persist.tile([1, NKV, L], BF16)          # sigmoid(x@Wl+bl), partition 0
        bl_sb = persist.tile([128, 1], F32)
        mask_sb = persist.tile([128, 256], BF16)
        onesf_sb = persist.tile([128, 128], F32R)
        onesb_sb = persist.tile([128, 1], BF16)

        xTr = xT.rearrange("(dc p) l -> p dc l", p=128)
        wqkr = Wqk.rearrange("(dc p) c -> p dc c", p=128)

        # ================= Phase 1: projections + RoPE =================
        with tc.tile_pool(name="ph1", bufs=1) as ph1, \
                tc.tile_pool(name="ps1", bufs=1, space="PSUM") as ps1:

            # startup-critical loads first, on the sync queue
            wt0 = ph1.tile([128, DC, 128], BF16, name="wt", tag="wt", bufs=3)
            nc.sync.dma_start(wt0[:], wqkr[:, :, 0:128])
            xs0 = ph1.tile([128, DC, 512], BF16, name="xs", tag="xs", bufs=2)
            nc.sync.dma_start(xs0[:, 0:2, :], xTr[:, 0:2, 0:512])
            nc.sync.dma_start(xs0[:, 2:DC, :], xTr[:, 2:DC, 0:512])
            # bulk loads ride behind on the gpsimd queue
            nc.gpsimd.dma_start(cos_sb[:], cosT[:, :])
            nc.gpsimd.dma_start(sin_sb[:], sinTs[:, :])
            nc.gpsimd.dma_start(wv_sb[:], Wv.rearrange("(dc p) c -> p dc c", p=128))
            nc.gpsimd.dma_start(wl_sb[:], Wl.rearrange("(dc p) c -> p dc c", p=128))
            nc.gpsimd.dma_start(bl_sb[:], blv[:, :])
            nc.gpsimd.dma_start(mask_sb[:], maskT[:, :])
            nc.gpsimd.dma_start(onesf_sb[:], onesin[:, :])
            nc.gpsimd.dma_start(onesb_sb[:], onesb[:, :])
            nc.gpsimd.dma_start(wo_sb[:], Wo.rearrange("(p d) o -> d p o", d=128))

            for lch in range(LCH):
                ls = slice(lch * 512, (lch + 1) * 512)
                if lch == 0:
                    xs = xs0
                else:
                    xs = ph1.tile([128, DC, 512], BF16, name="xs", tag="xs", bufs=2)
                    nc.sync.dma_start(xs[:], xTr[:, :, ls])

                # --- Q^T / K^T column tiles + RoPE ---
                for ct in range(CT):
                    if lch == 0 and ct == 0:
                        wt = wt0
                    else:
                        wt = ph1.tile([128, DC, 128], BF16, name="wt", tag="wt", bufs=3)
                        nc.sync.dma_start(wt[:], wqkr[:, :, ct * 128:(ct + 1) * 128])
                    qk_ps = ps1.tile([128, 512], F32, name="qk_ps", tag="mmq", bufs=4)
                    for dc in range(DC):
                        nc.tensor.matmul(
                            qk_ps[:], wt[:, dc, :], xs[:, dc, :],
                            start=(dc == 0), stop=(dc == DC - 1),
                        )
                    # RoPE: qr = qk*cos + rot(qk)*sin_signed
                    qf = ph1.tile([128, 512], BF16, name="qf", tag="qf", bufs=3)
                    nc.scalar.copy(qf[:], qk_ps[:])
                    rot = ph1.tile([128, 512], BF16, name="rot", tag="rot", bufs=3)
                    nc.gpsimd.dma_start(rot[0:64, :], qf[64:128, :])
                    nc.gpsimd.dma_start(rot[64:128, :], qf[0:64, :])
                    t1 = ph1.tile([128, 512], BF16, name="t1", tag="t1", bufs=2)
                    nc.vector.tensor_mul(t1[:], qf[:], cos_sb[:, ls])
                    t2 = ph1.tile([128, 512], BF16, name="t2", tag="t2", bufs=2)
                    nc.vector.tensor_mul(t2[:], rot[:], sin_sb[:, ls])
                    if ct < NQ:
                        dst = qres[:, lch * 4:(lch + 1) * 4, ct, :]
                        nc.vector.tensor_add(
                            dst,
                            t1.rearrange("p (t l) -> p t l", t=4),
                            t2.rearrange("p (t l) -> p t l", t=4),
                        )
                    else:
                        nc.vector.tensor_add(kres[:, ct - NQ, ls], t1[:], t2[:])

                # --- V tiles (l on partitions via x-as-stationary) ---
                for lt in range(4):
                    v_ps = ps1.tile([128, 512], F32, name="v_ps", tag="mmq", bufs=4)
                    for dc in range(DC):
                        nc.tensor.matmul(
                            v_ps[:], xs[:, dc, lt * 128:(lt + 1) * 128], wv_sb[:, dc, :],
                            start=(dc == 0), stop=(dc == DC - 1),
                        )
                    nc.scalar.copy(
                        vres[:, lch * 4 + lt, :, :].rearrange("p h d -> p (h d)"),
                        v_ps[:],
                    )

                # --- lambda ---
                lam_ps = ps1.tile([128, 512], F32, name="lam_ps", tag="mmq", bufs=4)
                for dc in range(DC):
                    nc.tensor.matmul(
                        lam_ps[:], wl_sb[:, dc, :], xs[:, dc, :],
                        start=(dc == 0), stop=(dc == DC - 1),
                    )
                lam4 = ph1.tile([NKV, 512], F32, name="lam4", tag="lam4", bufs=2)
                nc.scalar.activation(
                    lam4[:], lam_ps[0:NKV, :],
                    mybir.ActivationFunctionType.Sigmoid, bias=bl_sb[0:NKV, 0:1],
                )
                nc.gpsimd.dma_start(lamT[0:1, :, ls], lam4[:])

        # ============ Phase 2+3: causal attention + output projection ============
        # Per (sb, p) unit: S^T = K^T q over k-chunks 0..2sb+1, exp on ACT,
        # rowsum accumulated on DVE into rs_acc, ctx matmuls accumulate in PSUM.
        # Norm chains and Wo-projection quartets are deferred into a filler
        # queue and emitted between S-matmul groups to fill PE bubbles.
        with tc.tile_pool(name="ph2", bufs=1) as ph2, \
                tc.tile_pool(name="ps2", bufs=1, space="PSUM") as ps2:

            fillers = deque()

            def emit_fillers(n):
                for _ in range(min(n, len(fillers))):
                    fillers.popleft()()

            def make_norm(p, sb, rs_ps, ctxc):
                qtA = 2 * sb

                def norm():
                    recip = ph2.tile([1, 512], F32, name="recip", tag="recip", bufs=2)
                    nc.vector.reciprocal_approx_fast(recip[:], rs_ps[0:1, :])
                    r4 = recip.rearrange("p (t h l) -> p t h l", t=2, h=2)
                    cs = ph2.tile([1, 2, 2, 128], F32R, name="cs", tag="cs", bufs=2)
                    nc.vector.tensor_copy(cs[:, :, 0, :], r4[:, :, 0, :])
                    nc.vector.tensor_mul(
                        cs[:, :, 1, :], r4[:, :, 1, :],
                        lamT[0:1, p, qtA * 128:(qtA + 2) * 128].rearrange(
                            "p (t l) -> p t l", t=2
                        ),
                    )
                    b_ps = ps2.tile([128, 512], F32, name="b_ps", tag="ob", bufs=2)
                    nc.tensor.matmul(
                        b_ps[:], onesf_sb[0:1, :],
                        cs.rearrange("p t h l -> p (t h l)"),
                        start=True, stop=True, skip_group_check=True,
                    )
                    u = ph2.tile([128, 2, 2, 128], BF16, name="u", tag="u", bufs=2)
                    nc.vector.tensor_mul(
                        u.rearrange("p t h l -> p (t h l)"), ctxc[:], b_ps[:]
                    )
                    nc.vector.tensor_sub(
                        diffT[:, p, sb * 256:(sb + 1) * 256].rearrange(
                            "p (t l) -> p t l", t=2
                        ),
                        u[:, :, 0, :], u[:, :, 1, :],
                    )
                return norm

            def make_oquartet(qch, ot):
                def oq():
                    o_ps = ps2.tile([128, 512], F32, name="o_ps", tag="ob", bufs=2)
                    for p in range(NKV):
                        nc.tensor.matmul(
                            o_ps[:],
                            wo_sb[:, p, ot * 128:(ot + 1) * 128],
                            diffT[:, p, qch * 512:(qch + 1) * 512],
                            start=(p == 0), stop=(p == NKV - 1),
                            skip_group_check=True,
                        )
                    o_sb = ph2.tile([128, 512], F32, name="o_sb", tag="osb", bufs=3)
                    nc.scalar.copy(o_sb[:], o_ps[:])
                    nc.sync.dma_start(
                        outT[ot * 128:(ot + 1) * 128, qch * 512:(qch + 1) * 512],
                        o_sb[:],
                    )
                return oq

            for sb in range(SB):
                qtA, qtB = 2 * sb, 2 * sb + 1
                for p in range(NKV):
                    ctx_ps = ps2.tile([128, 512], F32, name="ctx_ps", tag="ctx", bufs=2)
                    rs_acc = ph2.tile([128, 512], BF16, name="rs_acc", tag="rsa", bufs=2)
                    pend = deque()

                    def emit_block(st):
                        e_sb, j, kc, off, wid = st
                        nc.tensor.matmul(
                            ctx_ps[:, off:off + wid], vres[:, kc, p, :],
                            e_sb[:, j, off:off + wid],
                            start=(kc == 0), stop=(kc == qtB),
                            skip_group_check=True,
                        )
                        if kc == 0:
                            nc.vector.tensor_copy(rs_acc[:], e_sb[:, 0, :])
                        else:
                            nc.vector.tensor_add(
                                rs_acc[:, off:off + wid], rs_acc[:, off:off + wid],
                                e_sb[:, j, off:off + wid],
                            )

                    kcs = list(range(qtB + 1))
                    groups = [kcs[i:i + 2] for i in range(0, len(kcs), 2)]
                    for gk, grp in enumerate(groups):
                        s_ps = ps2.tile([128, 2, 512], F32, name="s_ps", tag="s2", bufs=2)
                        segs = []
                        for j, kc in enumerate(grp):
                            off, wid = (256, 256) if kc == qtB else (0, 512)
                            rhs = (qres[:, qtA:qtA + 2, 2 * p:2 * p + 2, :]
                                   if wid == 512
                                   else qres[:, qtB, 2 * p:2 * p + 2, :])
                            nc.tensor.matmul(
                                s_ps[:, j, off:off + wid],
                                kres[:, p, kc * 128:(kc + 1) * 128],
                                rhs,
                                start=True, stop=True, skip_group_check=True,
                            )
                            segs.append((j, kc, off, wid))
                        emit_fillers(1)
                        while len(pend) >= 2:
                            emit_block(pend.popleft())
                        e_sb = ph2.tile([128, 2, 512], BF16, name="e_sb", tag="e", bufs=4)
                        if len(segs) == 2 and segs[0][3] == 512 and segs[1][3] == 512:
                            nc.scalar.activation(
                                e_sb.rearrange("p a b -> p (a b)"),
                                s_ps.rearrange("p a b -> p (a b)"),
                                mybir.ActivationFunctionType.Exp, scale=SCALE,
                            )
                        else:
                            for j, kc, off, wid in segs:
                                nc.scalar.activation(
                                    e_sb[:, j, off:off + wid], s_ps[:, j, off:off + wid],
                                    mybir.ActivationFunctionType.Exp, scale=SCALE,
                                )
                        for j, kc, off, wid in segs:
                            if kc == qtA:
                                nc.vector.tensor_mul(
                                    e_sb[:, j, 0:256], e_sb[:, j, 0:256], mask_sb[:]
                                )
                            elif kc == qtB:
                                nc.vector.tensor_mul(
                                    e_sb[:, j, 256:512], e_sb[:, j, 256:512], mask_sb[:]
                                )
                            pend.append((e_sb, j, kc, off, wid))
                    for st in pend:
                        emit_block(st)
                    # rowsum partition-reduction + eager ctx copy to SBUF
                    rs_ps = ps2.tile([128, 512], F32, name="rs_ps", tag="ob", bufs=2)
                    nc.tensor.matmul(
                        rs_ps[0:1, :], onesb_sb[:, 0:1], rs_acc[:],
                        start=True, stop=True, skip_group_check=True,
                    )
                    ctxc = ph2.tile([128, 512], BF16, name="ctxc", tag="ctxc", bufs=2)
                    nc.vector.tensor_copy(ctxc[:], ctx_ps[:])
                    fillers.appendleft(make_norm(p, sb, rs_ps, ctxc))
                if sb % 2 == 1:
                    qch = sb // 2
                    for ot in range(LT):
                        fillers.append(make_oquartet(qch, ot))
            while fillers:
                fillers.popleft()()

    nc.finalize()
    return nc


def _host_tables():
    half = DH // 2
    inv_freq = 1.0 / (ROPE_BASE ** (np.arange(0, half, dtype=np.float64) * 2.0 / DH))
    freqs = np.arange(L, dtype=np.float64)[:, None] * inv_freq[None, :]  # [L, half]
    emb = np.concatenate([freqs, freqs], axis=-1)  # [L, DH]
    cosT = np.ascontiguousarray(np.cos(emb).T.astype(np.float32))  # [DH, L]
    sinT = np.sin(emb).T.astype(np.float32)
    sinTs = np.concatenate([-sinT[:half], sinT[half:]], axis=0)
    sinTs = np.ascontiguousarray(sinTs.astype(np.float32))
    tri = np.triu(np.ones((128, 128), dtype=np.float32))  # keep k' <= q'
    maskT = np.ascontiguousarray(np.concatenate([tri, tri], axis=1))
    ones = np.ones((128, 128), dtype=np.float32)
    return cosT, sinTs, maskT, ones


_NC_CACHE = []


def kernel(x, Wq, Wk, Wv, Wl, bl, Wo):
    bf16 = ml_dtypes.bfloat16
    x = np.asarray(x, dtype=np.float32)
    Wq = np.asarray(Wq, dtype=np.float32)
    Wk = np.asarray(Wk, dtype=np.float32)
    Wv = np.asarray(Wv, dtype=np.float32)
    Wl = np.asarray(Wl, dtype=np.float32)
    bl = np.asarray(bl, dtype=np.float32)
    Wo = np.asarray(Wo, dtype=np.float32)

    cosT, sinTs, maskT, ones = _host_tables()
    Wq3 = Wq.reshape(D, 2 * NH, DH)
    Wk3 = Wk.reshape(D, NH, DH)

    in_maps = []
    for c in range(8):
        b, g = divmod(c, G)
        wq_s = Wq3[:, 8 * g:8 * g + NQ, :].reshape(D, NQ * DH)
        wk_s = Wk3[:, G * g:G * g + NKV, :].reshape(D, NKV * DH)
        in_maps.append({
            "xT": np.ascontiguousarray(x[b].T).astype(bf16),
            "Wqk": np.ascontiguousarray(
                np.concatenate([wq_s, wk_s], axis=1)).astype(bf16),
            "Wv": np.ascontiguousarray(
                Wv[:, DH * G * g:DH * G * g + NKV * DH]).astype(bf16),
            "Wl": np.ascontiguousarray(
                np.pad(Wl[:, G * g:G * g + NKV], ((0, 0), (0, 128 - NKV)))
            ).astype(bf16),
            "blv": np.ascontiguousarray(
                np.pad(bl[G * g:G * g + NKV], (0, 128 - NKV)).reshape(128, 1)),
            "Wo": np.ascontiguousarray(Wo[512 * g:512 * (g + 1), :]).astype(bf16),
            "cosT": cosT.astype(bf16),
            "sinTs": sinTs.astype(bf16),
            "maskT": maskT.astype(bf16),
            "onesin": ones,
            "onesb": np.ones((128, 1), dtype=np.float32).astype(bf16),
        })

    if not _NC_CACHE:
        _NC_CACHE.append(build_kernel())
    nc = _NC_CACHE[0]
    res = run_bass_kernel_spmd(nc, in_maps, core_ids=list(range(8)))

    out = np.empty((B, L, D), dtype=np.float32)
    for b in range(B):
        acc = res.results[4 * b]["outT"].copy()
        for g in range(1, G):
            acc += res.results[4 * b + g]["outT"]
        out[b] = acc.T
    return out


# revision 14
# speedup vs baseline: 1.2390x; 1.0372x over previous
"""DifferentialCausalAttention on 8 Trainium2 NeuronCores.

Sharding: 8 cores = 2 batches x 4 head-groups (tensor-parallel over heads).
Core c handles batch b = c // 4 and head-group g = c % 4:
  - query heads 8g..8g+7 (4 pairs), kv heads 4g..4g+3, lambda cols 4g..4g+3
  - W_O rows 512g..512g+511 -> partial output, host-summed over the 4 groups.

v2 design (vs baseline):
  - Q^T/K^T/V/diffT stay SBUF-resident between phases (no DRAM round-trip).
  - RoPE: one ACT copy PSUM->SBUF, partition-rotation via SBUF-SBUF DMA on the
    gpsimd queue, then bf16 DVE mul/mul/add (2x mode).
  - Attention rowsum moved off the PE: DVE accumulates exp tiles into rs_acc
    (bf16), one ones-matmul per (head, superblock) reduces partitions.
  - Output projection (Wo) matmuls are interleaved into the attention loop as
    filler work per 512-wide L chunk, so PE bubbles from the S->exp->ctx
    dependency chain are filled and there is no separate phase-3 window.
  - Startup DMA ordering: first weight tile + first x chunk land before the
    bulk loads, so the first matmul issues within a few us.
"""
import os
from collections import deque
from contextlib import ExitStack

import ml_dtypes
import numpy as np

import concourse.bass as bass
import concourse.mybir as mybir
import concourse.tile as tile
from concourse import bacc
from concourse.bass_utils import run_bass_kernel_spmd

F32 = mybir.dt.float32
F32R = mybir.dt.float32r
BF16 = mybir.dt.bfloat16

B, L, D, NH = 2, 2048, 2048, 16
DH = D // NH            # 128
G = 4                   # head groups (cores per batch)
NKV = NH // G           # kv heads per core = 4
NQ = 2 * NKV            # query heads per core = 8
CQK = NQ * DH + NKV * DH  # 1536 projection cols (Q then K)
CT = CQK // 128         # 12 column tiles (0-7 Q heads, 8-11 K heads)
DC = D // 128           # 16 contraction chunks
LCH = L // 512          # 4 L-chunks
LT = L // 128           # 16 L-tiles / q-tiles
SB = LT // 2            # 8 superblocks
SCALE = 1.0 / float(np.sqrt(DH))
ROPE_BASE = 10000.0


def build_kernel() -> bacc.Bacc:
    nc = bacc.Bacc("TRN2", target_bir_lowering=False, debug=False)

    # pre-tiled on host so every DMA is contiguous per partition
    xP = nc.dram_tensor("xP", [LCH, 128, DC, 512], BF16, kind="ExternalInput")
    WqkP = nc.dram_tensor("WqkP", [CT, 128, DC, 128], BF16, kind="ExternalInput")
    WvP = nc.dram_tensor("WvP", [128, DC, NKV * DH], BF16, kind="ExternalInput")
    WlP = nc.dram_tensor("WlP", [128, DC, 128], BF16, kind="ExternalInput")
    blv = nc.dram_tensor("blv", [128, 1], F32, kind="ExternalInput")
    Wo = nc.dram_tensor("Wo", [NKV * DH, D], BF16, kind="ExternalInput")
    cosT = nc.dram_tensor("cosT", [DH, L], BF16, kind="ExternalInput")
    sinTs = nc.dram_tensor("sinTs", [DH, L], BF16, kind="ExternalInput")
    maskT = nc.dram_tensor("maskT", [128, 256], BF16, kind="ExternalInput")
    onesin = nc.dram_tensor("onesin", [128, 128], F32R, kind="ExternalInput")
    onesb = nc.dram_tensor("onesb", [128, 1], BF16, kind="ExternalInput")
    outT = nc.dram_tensor("outT", [D, L], F32, kind="ExternalOutput")

    with ExitStack() as ctx:
        tc = ctx.enter_context(tile.TileContext(nc))

        persist = ctx.enter_context(tc.tile_pool(name="persist", bufs=1))

        # ---- persistent SBUF tensors ----
        qres = persist.tile([128, LT, NQ, 128], BF16)   # Q^T roped, (t, h, l)
        kres = persist.tile([128, NKV, L], BF16)        # K^T roped
        vres = persist.tile([128, LT, NKV, 128], BF16)  # V, l on partitions
        diffT = persist.tile([128, NKV, L], BF16)       # (ctx0-lam*ctx1)/rs ^T
        wo_sb = persist.tile([128, NKV, D], BF16)
        wv_sb = persist.tile([128, DC, NKV * DH], BF16)
        wl_sb = persist.tile([128, DC, 128], BF16)
        cos_sb = persist.tile([128, L], BF16)
        sin_sb = persist.tile([128, L], BF16)
        lamT = persist.tile([1, NKV, L], BF16)          # sigmoid(x@Wl+bl), partition 0
        bl_sb = persist.tile([128, 1], F32)
        mask_sb = persist.tile([128, 256], BF16)
        onesf_sb = persist.tile([128, 128], F32R)
        onesb_sb = persist.tile([128, 1], BF16)

        # ================= Phase 1: projections + RoPE =================
        with tc.tile_pool(name="ph1", bufs=1) as ph1, \
                tc.tile_pool(name="ps1", bufs=1, space="PSUM") as ps1:

            # startup-critical loads first, on the sync queue
            wt0 = ph1.tile([128, DC, 128], BF16, name="wt", tag="wt", bufs=3)
            nc.sync.dma_start(wt0[:], WqkP[0])
            xs0 = ph1.tile([128, DC, 512], BF16, name="xs", tag="xs", bufs=2)
            nc.sync.dma_start(xs0[:], xP[0])
            # bulk loads ride behind on the gpsimd queue
            nc.gpsimd.dma_start(cos_sb[:], cosT[:, :])
            nc.gpsimd.dma_start(sin_sb[:], sinTs[:, :])
            nc.gpsimd.dma_start(wv_sb[:], WvP[:])
            nc.gpsimd.dma_start(wl_sb[:], WlP[:])
            nc.gpsimd.dma_start(bl_sb[:], blv[:, :])
            nc.gpsimd.dma_start(mask_sb[:], maskT[:, :])
            nc.gpsimd.dma_start(onesf_sb[:], onesin[:, :])
            nc.gpsimd.dma_start(onesb_sb[:], onesb[:, :])
            nc.gpsimd.dma_start(wo_sb[:], Wo.rearrange("(p d) o -> d p o", d=128))

            for lch in range(LCH):
                ls = slice(lch * 512, (lch + 1) * 512)
                if lch == 0:
                    xs = xs0
                else:
                    xs = ph1.tile([128, DC, 512], BF16, name="xs", tag="xs", bufs=2)
                    nc.sync.dma_start(xs[:], xP[lch])

                # --- Q^T / K^T column tiles + RoPE ---
                for ct in range(CT):
                    if lch == 0 and ct == 0:
                        wt = wt0
                    else:
                        wt = ph1.tile([128, DC, 128], BF16, name="wt", tag="wt", bufs=3)
                        nc.sync.dma_start(wt[:], WqkP[ct])
                    qk_ps = ps1.tile([128, 512], F32, name="qk_ps", tag="mmq", bufs=4)
                    for dc in range(DC):
                        nc.tensor.matmul(
                            qk_ps[:], wt[:, dc, :], xs[:, dc, :],
                            start=(dc == 0), stop=(dc == DC - 1),
                        )
                    # RoPE: qr = qk*cos + rot(qk)*sin_signed
                    qf = ph1.tile([128, 512], BF16, name="qf", tag="qf", bufs=3)
                    nc.scalar.copy(qf[:], qk_ps[:])
                    rot = ph1.tile([128, 512], BF16, name="rot", tag="rot", bufs=3)
                    nc.gpsimd.dma_start(rot[0:64, :], qf[64:128, :])
                    nc.gpsimd.dma_start(rot[64:128, :], qf[0:64, :])
                    t1 = ph1.tile([128, 512], BF16, name="t1", tag="t1", bufs=2)
                    nc.vector.tensor_mul(t1[:], qf[:], cos_sb[:, ls])
                    t2 = ph1.tile([128, 512], BF16, name="t2", tag="t2", bufs=2)
                    nc.vector.tensor_mul(t2[:], rot[:], sin_sb[:, ls])
                    if ct < NQ:
                        dst = qres[:, lch * 4:(lch + 1) * 4, ct, :]
                        nc.vector.tensor_add(
                            dst,
                            t1.rearrange("p (t l) -> p t l", t=4),
                            t2.rearrange("p (t l) -> p t l", t=4),
                        )
                    else:
                        nc.vector.tensor_add(kres[:, ct - NQ, ls], t1[:], t2[:])

                # --- V tiles (l on partitions via x-as-stationary) ---
                for lt in range(4):
                    v_ps = ps1.tile([128, 512], F32, name="v_ps", tag="mmq", bufs=4)
                    for dc in range(DC):
                        nc.tensor.matmul(
                            v_ps[:], xs[:, dc, lt * 128:(lt + 1) * 128], wv_sb[:, dc, :],
                            start=(dc == 0), stop=(dc == DC - 1),
                        )
                    nc.scalar.copy(
                        vres[:, lch * 4 + lt, :, :].rearrange("p h d -> p (h d)"),
                        v_ps[:],
                    )

                # --- lambda ---
                lam_ps = ps1.tile([128, 512], F32, name="lam_ps", tag="mmq", bufs=4)
                for dc in range(DC):
                    nc.tensor.matmul(
                        lam_ps[:], wl_sb[:, dc, :], xs[:, dc, :],
                        start=(dc == 0), stop=(dc == DC - 1),
                    )
                lam4 = ph1.tile([NKV, 512], F32, name="lam4", tag="lam4", bufs=2)
                nc.scalar.activation(
                    lam4[:], lam_ps[0:NKV, :],
                    mybir.ActivationFunctionType.Sigmoid, bias=bl_sb[0:NKV, 0:1],
                )
                nc.gpsimd.dma_start(lamT[0:1, :, ls], lam4[:])

        # ============ Phase 2+3: causal attention + output projection ============
        # Per (sb, p) unit: S^T = K^T q over k-chunks 0..2sb+1, exp on ACT,
        # rowsum accumulated on DVE into rs_acc, ctx matmuls accumulate in PSUM.
        # Norm chains and Wo-projection quartets are deferred into a filler
        # queue and emitted between S-matmul groups to fill PE bubbles.
        with tc.tile_pool(name="ph2", bufs=1) as ph2, \
                tc.tile_pool(name="ps2", bufs=1, space="PSUM") as ps2:

            fillers = deque()

            def emit_fillers(n):
                for _ in range(min(n, len(fillers))):
                    fillers.popleft()()

            def make_norm(p, sb, rs_ps, ctxc):
                qtA = 2 * sb

                def norm():
                    recip = ph2.tile([1, 512], F32, name="recip", tag="recip", bufs=2)
                    nc.vector.reciprocal_approx_fast(recip[:], rs_ps[0:1, :])
                    r4 = recip.rearrange("p (t h l) -> p t h l", t=2, h=2)
                    cs = ph2.tile([1, 2, 2, 128], F32R, name="cs", tag="cs", bufs=2)
                    nc.vector.tensor_copy(cs[:, :, 0, :], r4[:, :, 0, :])
                    nc.vector.tensor_mul(
                        cs[:, :, 1, :], r4[:, :, 1, :],
                        lamT[0:1, p, qtA * 128:(qtA + 2) * 128].rearrange(
                            "p (t l) -> p t l", t=2
                        ),
                    )
                    b_ps = ps2.tile([128, 512], F32, name="b_ps", tag="ob", bufs=2)
                    nc.tensor.matmul(
                        b_ps[:], onesf_sb[0:1, :],
                        cs.rearrange("p t h l -> p (t h l)"),
                        start=True, stop=True, skip_group_check=True,
                    )
                    u = ph2.tile([128, 2, 2, 128], BF16, name="u", tag="u", bufs=2)
                    nc.vector.tensor_mul(
                        u.rearrange("p t h l -> p (t h l)"), ctxc[:], b_ps[:]
                    )
                    nc.vector.tensor_sub(
                        diffT[:, p, sb * 256:(sb + 1) * 256].rearrange(
                            "p (t l) -> p t l", t=2
                        ),
                        u[:, :, 0, :], u[:, :, 1, :],
                    )
                return norm

            def make_oquartet(qch, ot):
                def oq():
                    o_ps = ps2.tile([128, 512], F32, name="o_ps", tag="ob", bufs=2)
                    for p in range(NKV):
                        nc.tensor.matmul(
                            o_ps[:],
                            wo_sb[:, p, ot * 128:(ot + 1) * 128],
                            diffT[:, p, qch * 512:(qch + 1) * 512],
                            start=(p == 0), stop=(p == NKV - 1),
                            skip_group_check=True,
                        )
                    o_sb = ph2.tile([128, 512], F32, name="o_sb", tag="osb", bufs=4)
                    if ot % 2 == 0:
                        nc.scalar.copy(o_sb[:], o_ps[:])
                    else:
                        nc.vector.tensor_copy(o_sb[:], o_ps[:])
                    nc.sync.dma_start(
                        outT[ot * 128:(ot + 1) * 128, qch * 512:(qch + 1) * 512],
                        o_sb[:],
                    )
                return oq

            # big superblocks first: deep PE pipelines from the start, and the
            # Wo filler work for each L chunk becomes available early; the
            # final (smallest) superblocks leave only a short tail
            for sb in range(SB - 1, -1, -1):
                qtA, qtB = 2 * sb, 2 * sb + 1
                for p in range(NKV):
                    ctx_ps = ps2.tile([128, 512], F32, name="ctx_ps", tag="ctx", bufs=2)
                    rs_acc = ph2.tile([128, 512], BF16, name="rs_acc", tag="rsa", bufs=2)
                    pend = deque()

                    def emit_block(st):
                        e_sb, j, kc, off, wid = st
                        nc.tensor.matmul(
                            ctx_ps[:, off:off + wid], vres[:, kc, p, :],
                            e_sb[:, j, off:off + wid],
                            start=(kc == 0), stop=(kc == qtB),
                            skip_group_check=True,
                        )
                        if kc == 0:
                            nc.vector.tensor_copy(rs_acc[:], e_sb[:, 0, :])
                        else:
                            nc.vector.tensor_add(
                                rs_acc[:, off:off + wid], rs_acc[:, off:off + wid],
                                e_sb[:, j, off:off + wid],
                            )

                    kcs = list(range(qtB + 1))
                    groups = [kcs[i:i + 2] for i in range(0, len(kcs), 2)]
                    for gk, grp in enumerate(groups):
                        s_ps = ps2.tile([128, 2, 512], F32, name="s_ps", tag="s2", bufs=2)
                        segs = []
                        for j, kc in enumerate(grp):
                            off, wid = (256, 256) if kc == qtB else (0, 512)
                            rhs = (qres[:, qtA:qtA + 2, 2 * p:2 * p + 2, :]
                                   if wid == 512
                                   else qres[:, qtB, 2 * p:2 * p + 2, :])
                            nc.tensor.matmul(
                                s_ps[:, j, off:off + wid],
                                kres[:, p, kc * 128:(kc + 1) * 128],
                                rhs,
                                start=True, stop=True, skip_group_check=True,
                            )
                            segs.append((j, kc, off, wid))
                        emit_fillers(1)
                        while len(pend) >= 2:
                            emit_block(pend.popleft())
                        e_sb = ph2.tile([128, 2, 512], BF16, name="e_sb", tag="e", bufs=4)
                        if len(segs) == 2 and segs[0][3] == 512 and segs[1][3] == 512:
                            nc.scalar.activation(
                                e_sb.rearrange("p a b -> p (a b)"),
                                s_ps.rearrange("p a b -> p (a b)"),
                                mybir.ActivationFunctionType.Exp, scale=SCALE,
                            )
                        else:
                            for j, kc, off, wid in segs:
                                nc.scalar.activation(
                                    e_sb[:, j, off:off + wid], s_ps[:, j, off:off + wid],
                                    mybir.ActivationFunctionType.Exp, scale=SCALE,
                                )
                        for j, kc, off, wid in segs:
                            if kc == qtA:
                                nc.vector.tensor_mul(
                                    e_sb[:, j, 0:256], e_sb[:, j, 0:256], mask_sb[:]
                                )
                            elif kc == qtB:
                                nc.vector.tensor_mul(
                                    e_sb[:, j, 256:512], e_sb[:, j, 256:512], mask_sb[:]
                                )
                            pend.append((e_sb, j, kc, off, wid))
                    for st in pend:
                        emit_block(st)
                    # rowsum partition-reduction + eager ctx copy to SBUF
                    rs_ps = ps2.tile([128, 512], F32, name="rs_ps", tag="ob", bufs=2)
                    nc.tensor.matmul(
                        rs_ps[0:1, :], onesb_sb[:, 0:1], rs_acc[:],
                        start=True, stop=True, skip_group_check=True,
                    )
                    ctxc = ph2.tile([128, 512], BF16, name="ctxc", tag="ctxc", bufs=2)
                    nc.vector.tensor_copy(ctxc[:], ctx_ps[:])
                    fillers.appendleft(make_norm(p, sb, rs_ps, ctxc))
                if sb % 2 == 0:
                    qch = sb // 2
                    for ot in range(LT):
                        fillers.append(make_oquartet(qch, ot))
            while fillers:
                fillers.popleft()()

    nc.finalize()
    return nc


def _host_tables():
    half = DH // 2
    inv_freq = 1.0 / (ROPE_BASE ** (np.arange(0, half, dtype=np.float64) * 2.0 / DH))
    freqs = np.arange(L, dtype=np.float64)[:, None] * inv_freq[None, :]  # [L, half]
    emb = np.concatenate([freqs, freqs], axis=-1)  # [L, DH]
    cosT = np.ascontiguousarray(np.cos(emb).T.astype(np.float32))  # [DH, L]
    sinT = np.sin(emb).T.astype(np.float32)
    sinTs = np.concatenate([-sinT[:half], sinT[half:]], axis=0)
    sinTs = np.ascontiguousarray(sinTs.astype(np.float32))
    tri = np.triu(np.ones((128, 128), dtype=np.float32))  # keep k' <= q'
    maskT = np.ascontiguousarray(np.concatenate([tri, tri], axis=1))
    ones = np.ones((128, 128), dtype=np.float32)
    return cosT, sinTs, maskT, ones


_NC_CACHE = []


def kernel(x, Wq, Wk, Wv, Wl, bl, Wo):
    bf16 = ml_dtypes.bfloat16
    x = np.asarray(x, dtype=np.float32)
    Wq = np.asarray(Wq, dtype=np.float32)
    Wk = np.asarray(Wk, dtype=np.float32)
    Wv = np.asarray(Wv, dtype=np.float32)
    Wl = np.asarray(Wl, dtype=np.float32)
    bl = np.asarray(bl, dtype=np.float32)
    Wo = np.asarray(Wo, dtype=np.float32)

    cosT, sinTs, maskT, ones = _host_tables()
    Wq3 = Wq.reshape(D, 2 * NH, DH)
    Wk3 = Wk.reshape(D, NH, DH)

    def tile_in(w, ncols):
        # [D, C] -> [C//128, 128, DC, 128]: per-partition-contiguous DMA layout
        t = w.reshape(DC, 128, ncols // 128, 128)
        return np.ascontiguousarray(t.transpose(2, 1, 0, 3)).astype(bf16)

    def tile_flat(w, ncols):
        # [D, C] -> [128, DC, C]: per-partition-contiguous, all cols together
        t = w.reshape(DC, 128, ncols)
        return np.ascontiguousarray(t.transpose(1, 0, 2)).astype(bf16)

    in_maps = []
    xPs = {}
    for b in range(B):
        t = x[b].T.reshape(DC, 128, LCH, 512)  # [dc, p, lch, l]
        xPs[b] = np.ascontiguousarray(t.transpose(2, 1, 0, 3)).astype(bf16)
    for c in range(8):
        b, g = divmod(c, G)
        wq_s = Wq3[:, 8 * g:8 * g + NQ, :].reshape(D, NQ * DH)
        wk_s = Wk3[:, G * g:G * g + NKV, :].reshape(D, NKV * DH)
        wv_s = Wv[:, DH * G * g:DH * G * g + NKV * DH]
        wl_s = np.pad(Wl[:, G * g:G * g + NKV], ((0, 0), (0, 128 - NKV)))
        in_maps.append({
            "xP": xPs[b],
            "WqkP": tile_in(np.concatenate([wq_s, wk_s], axis=1), CQK),
            "WvP": tile_flat(wv_s, NKV * DH),
            "WlP": tile_flat(wl_s, 128),
            "blv": np.ascontiguousarray(
                np.pad(bl[G * g:G * g + NKV], (0, 128 - NKV)).reshape(128, 1)),
            "Wo": np.ascontiguousarray(Wo[512 * g:512 * (g + 1), :]).astype(bf16),
            "cosT": cosT.astype(bf16),
            "sinTs": sinTs.astype(bf16),
            "maskT": maskT.astype(bf16),
            "onesin": ones,
            "onesb": np.ones((128, 1), dtype=np.float32).astype(bf16),
        })

    if not _NC_CACHE:
        _NC_CACHE.append(build_kernel())
    nc = _NC_CACHE[0]
    res = run_bass_kernel_spmd(nc, in_maps, core_ids=list(range(8)))

    out = np.empty((B, L, D), dtype=np.float32)
    for b in range(B):
        acc = res.results[4 * b]["outT"].copy()
        for g in range(1, G):
            acc += res.results[4 * b + g]["outT"]
        out[b] = acc.T
    return out


# revision 24
# speedup vs baseline: 1.2611x; 1.0179x over previous
"""DifferentialCausalAttention on 8 Trainium2 NeuronCores.

Sharding: 8 cores = 2 batches x 4 head-groups (tensor-parallel over heads).
Core c handles batch b = c // 4 and head-group g = c % 4:
  - query heads 8g..8g+7 (4 pairs), kv heads 4g..4g+3, lambda cols 4g..4g+3
  - W_O rows 512g..512g+511 -> partial output, host-summed over the 4 groups.

v2 design (vs baseline):
  - Q^T/K^T/V/diffT stay SBUF-resident between phases (no DRAM round-trip).
  - RoPE: one ACT copy PSUM->SBUF, partition-rotation via SBUF-SBUF DMA on the
    gpsimd queue, then bf16 DVE mul/mul/add (2x mode).
  - Attention rowsum moved off the PE: DVE accumulates exp tiles into rs_acc
    (bf16), one ones-matmul per (head, superblock) reduces partitions.
  - Output projection (Wo) matmuls are interleaved into the attention loop as
    filler work per 512-wide L chunk, so PE bubbles from the S->exp->ctx
    dependency chain are filled and there is no separate phase-3 window.
  - Startup DMA ordering: first weight tile + first x chunk land before the
    bulk loads, so the first matmul issues within a few us.
"""
import os
from collections import deque
from contextlib import ExitStack

import ml_dtypes
import numpy as np

import concourse.bass as bass
import concourse.mybir as mybir
import concourse.tile as tile
from concourse import bacc
from concourse.bass_utils import run_bass_kernel_spmd

F32 = mybir.dt.float32
F32R = mybir.dt.float32r
BF16 = mybir.dt.bfloat16

B, L, D, NH = 2, 2048, 2048, 16
DH = D // NH            # 128
G = 4                   # head groups (cores per batch)
NKV = NH // G           # kv heads per core = 4
NQ = 2 * NKV            # query heads per core = 8
CQK = NQ * DH + NKV * DH  # 1536 projection cols (Q then K)
CT = CQK // 128         # 12 column tiles (0-7 Q heads, 8-11 K heads)
DC = D // 128           # 16 contraction chunks
LCH = L // 512          # 4 L-chunks
LT = L // 128           # 16 L-tiles / q-tiles
SB = LT // 2            # 8 superblocks
SCALE = 1.0 / float(np.sqrt(DH))
ROPE_BASE = 10000.0


def build_kernel() -> bacc.Bacc:
    nc = bacc.Bacc("TRN2", target_bir_lowering=False, debug=False)

    # pre-tiled on host so every DMA is contiguous per partition
    xP = nc.dram_tensor("xP", [LCH, 128, DC, 512], BF16, kind="ExternalInput")
    WqkP = nc.dram_tensor("WqkP", [CT, 128, DC, 128], BF16, kind="ExternalInput")
    WvP = nc.dram_tensor("WvP", [128, DC, NKV * DH], BF16, kind="ExternalInput")
    WlP = nc.dram_tensor("WlP", [128, DC, 128], BF16, kind="ExternalInput")
    blv = nc.dram_tensor("blv", [128, 1], F32, kind="ExternalInput")
    Wo = nc.dram_tensor("Wo", [NKV * DH, D], BF16, kind="ExternalInput")
    cosT = nc.dram_tensor("cosT", [DH, L], BF16, kind="ExternalInput")
    sinTs = nc.dram_tensor("sinTs", [DH, L], BF16, kind="ExternalInput")
    maskT = nc.dram_tensor("maskT", [128, 256], BF16, kind="ExternalInput")
    onesin = nc.dram_tensor("onesin", [128, 128], F32R, kind="ExternalInput")
    onesb = nc.dram_tensor("onesb", [128, 1], BF16, kind="ExternalInput")
    outT = nc.dram_tensor("outT", [D, L], BF16, kind="ExternalOutput")

    with ExitStack() as ctx:
        tc = ctx.enter_context(tile.TileContext(nc))

        persist = ctx.enter_context(tc.tile_pool(name="persist", bufs=1))

        # ---- persistent SBUF tensors ----
        qres = persist.tile([128, LT, NQ, 128], BF16)   # Q^T roped, (t, h, l)
        kres = persist.tile([128, NKV, L], BF16)        # K^T roped
        vres = persist.tile([128, LT, NKV, 128], BF16)  # V, l on partitions
        diffT = persist.tile([128, NKV, L], BF16)       # (ctx0-lam*ctx1)/rs ^T
        wo_sb = persist.tile([128, NKV, D], BF16)
        wv_sb = persist.tile([128, DC, NKV * DH], BF16)
        wl_sb = persist.tile([128, DC, 128], BF16)
        cos_sb = persist.tile([128, L], BF16)
        sin_sb = persist.tile([128, L], BF16)
        lamT = persist.tile([1, NKV, L], BF16)          # sigmoid(x@Wl+bl), partition 0
        bl_sb = persist.tile([128, 1], F32)
        mask_sb = persist.tile([128, 256], BF16)
        onesf_sb = persist.tile([128, 128], F32R)
        onesb_sb = persist.tile([128, 1], BF16)

        # ================= Phase 1: projections + RoPE =================
        with tc.tile_pool(name="ph1", bufs=1) as ph1, \
                tc.tile_pool(name="ps1", bufs=1, space="PSUM") as ps1:

            # startup-critical loads first, on the sync queue; first matmul
            # only needs wt0 + the first x chunks, so split the x load
            wt0 = ph1.tile([128, DC, 128], BF16, name="wt", tag="wt", bufs=3)
            nc.sync.dma_start(wt0[:], WqkP[0])
            xs0 = ph1.tile([128, DC, 512], BF16, name="xs", tag="xs", bufs=2)
            nc.sync.dma_start(xs0[:, 0:2, :], xP[0, :, 0:2, :])
            nc.sync.dma_start(xs0[:, 2:DC, :], xP[0, :, 2:DC, :])
            # bulk loads ride behind on the gpsimd queue
            nc.gpsimd.dma_start(cos_sb[:], cosT[:, :])
            nc.gpsimd.dma_start(sin_sb[:], sinTs[:, :])
            nc.gpsimd.dma_start(wv_sb[:], WvP[:])
            nc.gpsimd.dma_start(wl_sb[:], WlP[:])
            nc.gpsimd.dma_start(bl_sb[:], blv[:, :])
            nc.gpsimd.dma_start(mask_sb[:], maskT[:, :])
            nc.gpsimd.dma_start(onesf_sb[:], onesin[:, :])
            nc.gpsimd.dma_start(onesb_sb[:], onesb[:, :])
            nc.gpsimd.dma_start(wo_sb[:], Wo.rearrange("(p d) o -> d p o", d=128))

            xs = xs0
            for lch in range(LCH):
                ls = slice(lch * 512, (lch + 1) * 512)
                xs_next = None
                if lch + 1 < LCH:
                    xs_next = ph1.tile([128, DC, 512], BF16, name="xs", tag="xs", bufs=2)

                # --- Q^T / K^T column tiles + RoPE ---
                for ct in range(CT):
                    if lch == 0 and ct == 0:
                        wt = wt0
                    else:
                        wt = ph1.tile([128, DC, 128], BF16, name="wt", tag="wt", bufs=3)
                        nc.sync.dma_start(wt[:], WqkP[ct])
                    if ct == 3 and xs_next is not None:
                        # next-lch x prefetch, emitted here so the first weight
                        # tiles of this lch don't queue behind a 2MB transfer
                        nc.sync.dma_start(xs_next[:], xP[lch + 1])
                    qk_ps = ps1.tile([128, 512], F32, name="qk_ps", tag="mmq", bufs=4)
                    for dc in range(DC):
                        nc.tensor.matmul(
                            qk_ps[:], wt[:, dc, :], xs[:, dc, :],
                            start=(dc == 0), stop=(dc == DC - 1),
                        )
                    # RoPE: qr = qk*cos + rot(qk)*sin_signed
                    qf = ph1.tile([128, 512], BF16, name="qf", tag="qf", bufs=3)
                    nc.scalar.copy(qf[:], qk_ps[:])
                    rot = ph1.tile([128, 512], BF16, name="rot", tag="rot", bufs=3)
                    nc.gpsimd.dma_start(rot[0:64, :], qf[64:128, :])
                    nc.gpsimd.dma_start(rot[64:128, :], qf[0:64, :])
                    t1 = ph1.tile([128, 512], BF16, name="t1", tag="t1", bufs=2)
                    nc.vector.tensor_mul(t1[:], qf[:], cos_sb[:, ls])
                    t2 = ph1.tile([128, 512], BF16, name="t2", tag="t2", bufs=2)
                    nc.vector.tensor_mul(t2[:], rot[:], sin_sb[:, ls])
                    if ct < NQ:
                        dst = qres[:, lch * 4:(lch + 1) * 4, ct, :]
                        nc.vector.tensor_add(
                            dst,
                            t1.rearrange("p (t l) -> p t l", t=4),
                            t2.rearrange("p (t l) -> p t l", t=4),
                        )
                    else:
                        nc.vector.tensor_add(kres[:, ct - NQ, ls], t1[:], t2[:])

                # --- V tiles (l on partitions via x-as-stationary) ---
                for lt in range(4):
                    v_ps = ps1.tile([128, 512], F32, name="v_ps", tag="mmq", bufs=4)
                    for dc in range(DC):
                        nc.tensor.matmul(
                            v_ps[:], xs[:, dc, lt * 128:(lt + 1) * 128], wv_sb[:, dc, :],
                            start=(dc == 0), stop=(dc == DC - 1),
                        )
                    nc.scalar.copy(
                        vres[:, lch * 4 + lt, :, :].rearrange("p h d -> p (h d)"),
                        v_ps[:],
                    )

                # --- lambda ---
                lam_ps = ps1.tile([128, 512], F32, name="lam_ps", tag="mmq", bufs=4)
                for dc in range(DC):
                    nc.tensor.matmul(
                        lam_ps[:], wl_sb[:, dc, :], xs[:, dc, :],
                        start=(dc == 0), stop=(dc == DC - 1),
                    )
                lam4 = ph1.tile([NKV, 512], F32, name="lam4", tag="lam4", bufs=2)
                nc.scalar.activation(
                    lam4[:], lam_ps[0:NKV, :],
                    mybir.ActivationFunctionType.Sigmoid, bias=bl_sb[0:NKV, 0:1],
                )
                nc.gpsimd.dma_start(lamT[0:1, :, ls], lam4[:])
                xs = xs_next

        # ============ Phase 2+3: causal attention + output projection ============
        # Per (sb, p) unit: S^T = K^T q over k-chunks 0..2sb+1, exp on ACT,
        # rowsum accumulated on DVE into rs_acc, ctx matmuls accumulate in PSUM.
        # Norm chains and Wo-projection quartets are deferred into a filler
        # queue and emitted between S-matmul groups to fill PE bubbles.
        with tc.tile_pool(name="ph2", bufs=1) as ph2, \
                tc.tile_pool(name="ps2", bufs=1, space="PSUM") as ps2:

            fillers = deque()

            def emit_fillers(n):
                for _ in range(min(n, len(fillers))):
                    fillers.popleft()()

            def make_norm(p, sb, rs_ps, ctxc):
                qtA = 2 * sb

                def norm():
                    recip = ph2.tile([1, 512], F32, name="recip", tag="recip", bufs=2)
                    nc.vector.reciprocal_approx_fast(recip[:], rs_ps[0:1, :])
                    r4 = recip.rearrange("p (t h l) -> p t h l", t=2, h=2)
                    cs = ph2.tile([1, 2, 2, 128], F32R, name="cs", tag="cs", bufs=2)
                    nc.vector.tensor_copy(cs[:, :, 0, :], r4[:, :, 0, :])
                    nc.vector.tensor_mul(
                        cs[:, :, 1, :], r4[:, :, 1, :],
                        lamT[0:1, p, qtA * 128:(qtA + 2) * 128].rearrange(
                            "p (t l) -> p t l", t=2
                        ),
                    )
                    b_ps = ps2.tile([128, 512], F32, name="b_ps", tag="ob", bufs=2)
                    nc.tensor.matmul(
                        b_ps[:], onesf_sb[0:1, :],
                        cs.rearrange("p t h l -> p (t h l)"),
                        start=True, stop=True, skip_group_check=True,
                    )
                    u = ph2.tile([128, 2, 2, 128], BF16, name="u", tag="u", bufs=2)
                    nc.vector.tensor_mul(
                        u.rearrange("p t h l -> p (t h l)"), ctxc[:], b_ps[:]
                    )
                    nc.vector.tensor_sub(
                        diffT[:, p, sb * 256:(sb + 1) * 256].rearrange(
                            "p (t l) -> p t l", t=2
                        ),
                        u[:, :, 0, :], u[:, :, 1, :],
                    )
                return norm

            def make_oquartet(qch, ot):
                def oq():
                    o_ps = ps2.tile([128, 512], F32, name="o_ps", tag="ob", bufs=2)
                    for p in range(NKV):
                        nc.tensor.matmul(
                            o_ps[:],
                            wo_sb[:, p, ot * 128:(ot + 1) * 128],
                            diffT[:, p, qch * 512:(qch + 1) * 512],
                            start=(p == 0), stop=(p == NKV - 1),
                            skip_group_check=True,
                        )
                    o_sb = ph2.tile([128, 512], BF16, name="o_sb", tag="osb", bufs=4)
                    nc.scalar.copy(o_sb[:], o_ps[:])
                    nc.sync.dma_start(
                        outT[ot * 128:(ot + 1) * 128, qch * 512:(qch + 1) * 512],
                        o_sb[:],
                    )
                return oq

            # big superblocks first: deep PE pipelines from the start, and the
            # Wo filler work for each L chunk becomes available early; the
            # final (smallest) superblocks leave only a short tail
            for sb in range(SB - 1, -1, -1):
                qtA, qtB = 2 * sb, 2 * sb + 1
                for p in range(NKV):
                    ctx_ps = ps2.tile([128, 512], F32, name="ctx_ps", tag="ctx", bufs=2)
                    # two half-accumulators so each full group is ONE 1024-wide add
                    rs_acc = ph2.tile([128, 2, 512], BF16, name="rs_acc", tag="rsa", bufs=2)
                    pend = deque()

                    def emit_block(st):
                        e_sb, segs, first = st
                        for j, kc, off, wid in segs:
                            nc.tensor.matmul(
                                ctx_ps[:, off:off + wid], vres[:, kc, p, :],
                                e_sb[:, j, off:off + wid],
                                start=(kc == 0), stop=(kc == qtB),
                                skip_group_check=True,
                            )
                        full = (len(segs) == 2 and segs[0][3] == 512
                                and segs[1][3] == 512)
                        ef = e_sb.rearrange("p a b -> p (a b)")
                        rf = rs_acc.rearrange("p a b -> p (a b)")
                        if full:
                            if first:
                                nc.vector.tensor_copy(rf[:], ef[:])
                            else:
                                nc.vector.tensor_add(rf[:], rf[:], ef[:])
                        else:
                            for j, kc, off, wid in segs:
                                sl = (slice(None), j, slice(off, off + wid))
                                if first and j == 0:
                                    nc.vector.tensor_copy(rs_acc[sl], e_sb[sl])
                                elif first and j == 1:
                                    nc.vector.tensor_copy(rs_acc[sl], e_sb[sl])
                                else:
                                    nc.vector.tensor_add(
                                        rs_acc[sl], rs_acc[sl], e_sb[sl]
                                    )

                    kcs = list(range(qtB + 1))
                    groups = [kcs[i:i + 2] for i in range(0, len(kcs), 2)]
                    for gk, grp in enumerate(groups):
                        s_ps = ps2.tile([128, 2, 512], F32, name="s_ps", tag="s2", bufs=2)
                        segs = []
                        for j, kc in enumerate(grp):
                            off, wid = (256, 256) if kc == qtB else (0, 512)
                            rhs = (qres[:, qtA:qtA + 2, 2 * p:2 * p + 2, :]
                                   if wid == 512
                                   else qres[:, qtB, 2 * p:2 * p + 2, :])
                            nc.tensor.matmul(
                                s_ps[:, j, off:off + wid],
                                kres[:, p, kc * 128:(kc + 1) * 128],
                                rhs,
                                start=True, stop=True, skip_group_check=True,
                            )
                            segs.append((j, kc, off, wid))
                        emit_fillers(1)
                        while len(pend) >= 2:
                            emit_block(pend.popleft())
                        e_sb = ph2.tile([128, 2, 512], BF16, name="e_sb", tag="e", bufs=6)
                        if len(segs) == 2 and segs[0][3] == 512 and segs[1][3] == 512:
                            nc.scalar.activation(
                                e_sb.rearrange("p a b -> p (a b)"),
                                s_ps.rearrange("p a b -> p (a b)"),
                                mybir.ActivationFunctionType.Exp, scale=SCALE,
                            )
                        else:
                            for j, kc, off, wid in segs:
                                nc.scalar.activation(
                                    e_sb[:, j, off:off + wid], s_ps[:, j, off:off + wid],
                                    mybir.ActivationFunctionType.Exp, scale=SCALE,
                                )
                        for j, kc, off, wid in segs:
                            if kc == qtA:
                                nc.vector.tensor_mul(
                                    e_sb[:, j, 0:256], e_sb[:, j, 0:256], mask_sb[:]
                                )
                            elif kc == qtB:
                                nc.vector.tensor_mul(
                                    e_sb[:, j, 256:512], e_sb[:, j, 256:512], mask_sb[:]
                                )
                        pend.append((e_sb, segs, gk == 0))
                    for st in pend:
                        emit_block(st)
                    # rowsum partition-reduction (both halves into one psum row)
                    rs_ps = ps2.tile([128, 512], F32, name="rs_ps", tag="ob", bufs=2)
                    nc.tensor.matmul(
                        rs_ps[0:1, :], onesb_sb[:, 0:1], rs_acc[:, 0, :],
                        start=True, stop=False, skip_group_check=True,
                    )
                    w1 = 256 if sb == 0 else 512
                    nc.tensor.matmul(
                        rs_ps[0:1, 512 - w1:512], onesb_sb[:, 0:1],
                        rs_acc[:, 1, 512 - w1:512],
                        start=False, stop=True, skip_group_check=True,
                    )
                    ctxc = ph2.tile([128, 512], BF16, name="ctxc", tag="ctxc", bufs=2)
                    nc.vector.tensor_copy(ctxc[:], ctx_ps[:])
                    fillers.appendleft(make_norm(p, sb, rs_ps, ctxc))
                if sb % 2 == 0:
                    qch = sb // 2
                    for ot in range(LT):
                        fillers.append(make_oquartet(qch, ot))
            while fillers:
                fillers.popleft()()

    nc.finalize()
    return nc


def _host_tables():
    half = DH // 2
    inv_freq = 1.0 / (ROPE_BASE ** (np.arange(0, half, dtype=np.float64) * 2.0 / DH))
    freqs = np.arange(L, dtype=np.float64)[:, None] * inv_freq[None, :]  # [L, half]
    emb = np.concatenate([freqs, freqs], axis=-1)  # [L, DH]
    cosT = np.ascontiguousarray(np.cos(emb).T.astype(np.float32))  # [DH, L]
    sinT = np.sin(emb).T.astype(np.float32)
    sinTs = np.concatenate([-sinT[:half], sinT[half:]], axis=0)
    sinTs = np.ascontiguousarray(sinTs.astype(np.float32))
    tri = np.triu(np.ones((128, 128), dtype=np.float32))  # keep k' <= q'
    maskT = np.ascontiguousarray(np.concatenate([tri, tri], axis=1))
    ones = np.ones((128, 128), dtype=np.float32)
    return cosT, sinTs, maskT, ones


_NC_CACHE = []


def kernel(x, Wq, Wk, Wv, Wl, bl, Wo):
    bf16 = ml_dtypes.bfloat16
    x = np.asarray(x, dtype=np.float32)
    Wq = np.asarray(Wq, dtype=np.float32)
    Wk = np.asarray(Wk, dtype=np.float32)
    Wv = np.asarray(Wv, dtype=np.float32)
    Wl = np.asarray(Wl, dtype=np.float32)
    bl = np.asarray(bl, dtype=np.float32)
    Wo = np.asarray(Wo, dtype=np.float32)

    cosT, sinTs, maskT, ones = _host_tables()
    Wq3 = Wq.reshape(D, 2 * NH, DH)
    Wk3 = Wk.reshape(D, NH, DH)

    def tile_in(w, ncols):
        # [D, C] -> [C//128, 128, DC, 128]: per-partition-contiguous DMA layout
        t = w.reshape(DC, 128, ncols // 128, 128)
        return np.ascontiguousarray(t.transpose(2, 1, 0, 3)).astype(bf16)

    def tile_flat(w, ncols):
        # [D, C] -> [128, DC, C]: per-partition-contiguous, all cols together
        t = w.reshape(DC, 128, ncols)
        return np.ascontiguousarray(t.transpose(1, 0, 2)).astype(bf16)

    in_maps = []
    xPs = {}
    for b in range(B):
        t = x[b].T.reshape(DC, 128, LCH, 512)  # [dc, p, lch, l]
        xPs[b] = np.ascontiguousarray(t.transpose(2, 1, 0, 3)).astype(bf16)
    for c in range(8):
        b, g = divmod(c, G)
        wq_s = Wq3[:, 8 * g:8 * g + NQ, :].reshape(D, NQ * DH)
        wk_s = Wk3[:, G * g:G * g + NKV, :].reshape(D, NKV * DH)
        wv_s = Wv[:, DH * G * g:DH * G * g + NKV * DH]
        wl_s = np.pad(Wl[:, G * g:G * g + NKV], ((0, 0), (0, 128 - NKV)))
        in_maps.append({
            "xP": xPs[b],
            "WqkP": tile_in(np.concatenate([wq_s, wk_s], axis=1), CQK),
            "WvP": tile_flat(wv_s, NKV * DH),
            "WlP": tile_flat(wl_s, 128),
            "blv": np.ascontiguousarray(
                np.pad(bl[G * g:G * g + NKV], (0, 128 - NKV)).reshape(128, 1)),
            "Wo": np.ascontiguousarray(Wo[512 * g:512 * (g + 1), :]).astype(bf16),
            "cosT": cosT.astype(bf16),
            "sinTs": sinTs.astype(bf16),
            "maskT": maskT.astype(bf16),
            "onesin": ones,
            "onesb": np.ones((128, 1), dtype=np.float32).astype(bf16),
        })

    if not _NC_CACHE:
        _NC_CACHE.append(build_kernel())
    nc = _NC_CACHE[0]
    res = run_bass_kernel_spmd(nc, in_maps, core_ids=list(range(8)))

    out = np.empty((B, L, D), dtype=np.float32)
    for b in range(B):
        acc = res.results[4 * b]["outT"].astype(np.float32)
        for g in range(1, G):
            acc += res.results[4 * b + g]["outT"].astype(np.float32)
        out[b] = acc.T
    return out
